# revision 18
# baseline (speedup 1.0000x reference)
"""Trainium2 Bass kernel for nn_ContinuousEmbedding (masked matmul + bias).

Computes out = x @ (weights * mask) + bias, reshaped to [B, in_size, out_size],
where mask zeroes each input feature's own [out_size]-wide diagonal block.

Strategy: tensor-parallel across the 8 NeuronCores by splitting the
in_size*out_size (=16384) output columns into 8 shards of 2048 columns.
The mask is constant and folded into the weights on the host.

The kernel sits on the HBM-bandwidth / TensorE-stream ridge (~55 us each
per core), so both sides are minimized:
  - x^T and the weight shard are shipped as fp16 (exact-cast on host); the
    output shard is written back as fp16 and upcast to fp32 on the host.
    The bias add also happens on the host, after the upcast. End-to-end
    rel-l2 error ~3.6e-4 (fp16 mantissa), far inside the 2e-2 gate, while
    halving every DMA byte vs fp32: 19.9 MB/core instead of 40.9 MB.
  - per 128-row m-tile: 8 fp16 matmuls (2 K-tiles x 4 N-banks) accumulate
    into TWO 2-bank [128,1024] fp32 PSUM tiles from separate pools. The
    PSUM evictions are pure fp32->fp16 copies running CONCURRENTLY on the
    DVE (ps_l) and Activation (ps_r) engines into separate SBUF tiles.
    The splits matter: the Tile dependency tracker serializes two engines
    touching the same tile (even two readers), and a single evictor
    (~2.3 us/tile vs the 1.73 us matmul group) stalls the PE on PSUM
    reuse; with the split the PE streams gap-free at 216 ns/matmul.
  - loads ride the Activation HWDGE ring (W k=0, first x^T block, W k=1,
    rest of x^T in growing chunks) so m-tile 0's matmuls start right
    after the runtime preamble + DMA completion latency (~11.3 us);
    stores ride the SP ring and never queue behind loads.
"""

import numpy as np

B = 4096
IN_SIZE = 256
OUT_SIZE = 64
IO = IN_SIZE * OUT_SIZE          # 16384
N_CORES = 8
N_SHARD = IO // N_CORES          # 2048 output columns per core
P = 128                          # SBUF partitions
KO = IN_SIZE // P                # 2 contraction sub-tiles
N_TILE = 512                     # matmul moving free dim (= 1 PSUM bank fp32)
M_TILES = B // P                 # 32 output row tiles
N_TILES = N_SHARD // N_TILE      # 4 PSUM banks per m-tile

# x^T arrives in growing chunks so early m-tiles can start while the rest
# loads.
XT_CHUNKS = [(0, 256), (256, 768), (768, 1792), (1792, B)]

_CACHE: dict = {}


def _build_program():
    import concourse.mybir as mybir
    import concourse.tile as tile
    from concourse import bacc

    nc = bacc.Bacc(
        "TRN2", target_bir_lowering=False, debug=False, num_devices=N_CORES
    )
    dt16 = mybir.dt.float16
    xt = nc.dram_tensor("xt", [KO, P, B], dt16, kind="ExternalInput").ap()
    w = nc.dram_tensor("w", [KO, P, N_SHARD], dt16, kind="ExternalInput").ap()
    out = nc.dram_tensor(
        "out", [B, N_SHARD], dt16, kind="ExternalOutput"
    ).ap()

    with tile.TileContext(nc) as tc:
        with tc.tile_pool(name="const", bufs=1) as const, \
             tc.tile_pool(name="psl", bufs=2, space="PSUM") as psl_pool, \
             tc.tile_pool(name="psr", bufs=2, space="PSUM") as psr_pool, \
             tc.tile_pool(name="outl", bufs=6) as outl, \
             tc.tile_pool(name="outr", bufs=6) as outr:
            half = N_SHARD // 2
            w_sb = const.tile([P, KO, N_SHARD], dt16)
            xt_sb = const.tile([P, KO, B], dt16)

            # Load ring (ACT): W k=0, then m-tile 0's x^T block, then W
            # k=1, then the remaining x^T chunks — ordered so the first
            # matmul group issues as early as possible. (First-MM latency
            # is bounded by DMA completion-receipt latency, ~2 us past
            # last byte; finer chunking than this buys nothing.)
            w_src = w.rearrange("ko p n -> p ko n")
            xt_src = xt.rearrange("ko p m -> p ko m")
            nc.scalar.dma_start(out=w_sb[:, 0, :], in_=w_src[:, 0, :])
            lo, hi = XT_CHUNKS[0]
            nc.scalar.dma_start(
                out=xt_sb[:, :, lo:hi], in_=xt_src[:, :, lo:hi]
            )
            nc.scalar.dma_start(out=w_sb[:, 1, :], in_=w_src[:, 1, :])
            for lo, hi in XT_CHUNKS[1:]:
                sl = slice(lo, hi)
                nc.scalar.dma_start(out=xt_sb[:, :, sl], in_=xt_src[:, :, sl])

            for m in range(M_TILES):
                out_l = outl.tile([P, half], dt16)
                out_r = outr.tile([P, half], dt16)
                # Two PSUM tiles per m-tile: the dependency tracker
                # serializes two ENGINES touching the same tile (even two
                # readers), so DVE gets its own 2-bank tile and ACT the
                # other — evictions then truly run concurrently.
                ps_l = psl_pool.tile([P, half], mybir.dt.float32)
                ps_r = psr_pool.tile([P, half], mybir.dt.float32)
                ms = slice(m * P, (m + 1) * P)
                # k-outer so 4 consecutive matmuls share the stationary
                # x^T tile (single weight load into the PE per k).
                for k in range(KO):
                    for n in range(N_TILES):
                        ps = ps_l if n < N_TILES // 2 else ps_r
                        nn = n % (N_TILES // 2)
                        ns = slice(nn * N_TILE, (nn + 1) * N_TILE)
                        nc.tensor.matmul(
                            ps[:, ns],
                            lhsT=xt_sb[:, k, ms],
                            rhs=w_sb[:, k, n * N_TILE:(n + 1) * N_TILE],
                            start=(k == 0),
                            stop=(k == KO - 1),
                        )
                # PSUM evictions = pure fp32->fp16 copies on two engines
                # concurrently, each into its own SBUF tile.
                nc.vector.tensor_copy(out_l[:], ps_l[:])
                nc.scalar.copy(out_r[:], ps_r[:])
                nc.sync.dma_start(out=out[ms, :half], in_=out_l[:])
                nc.sync.dma_start(out=out[ms, half:], in_=out_r[:])

    nc.compile()
    return nc


def _get_program():
    if "nc" not in _CACHE:
        _CACHE["nc"] = _build_program()
    return _CACHE["nc"]


def _shard_inputs(x, weights, bias=None):
    # Fold the constant block-diagonal mask into the weights on the host.
    col_block = np.arange(IO, dtype=np.int64) // OUT_SIZE
    mask = (col_block[None, :] != np.arange(IN_SIZE)[:, None])
    wm = weights * mask.astype(weights.dtype)
    xt = np.ascontiguousarray(x.T.astype(np.float16)).reshape(KO, P, B)
    in_maps = []
    for c in range(N_CORES):
        sl = slice(c * N_SHARD, (c + 1) * N_SHARD)
        w_shard = np.ascontiguousarray(
            wm[:, sl].astype(np.float16)
        ).reshape(KO, P, N_SHARD)
        in_maps.append({"xt": xt, "w": w_shard})
    return in_maps


def run_sharded(in_maps, **kwargs):
    """Run the SPMD program on cores 0-7. kwargs forwarded (e.g. trace)."""
    from concourse.bass_utils import run_bass_kernel_spmd

    nc = _get_program()
    return run_bass_kernel_spmd(
        nc, in_maps, core_ids=list(range(N_CORES)), **kwargs
    )


def kernel(x: np.ndarray, weights: np.ndarray, bias: np.ndarray) -> np.ndarray:
    x = np.asarray(x, dtype=np.float32)
    weights = np.asarray(weights, dtype=np.float32)
    bias = np.asarray(bias, dtype=np.float32)
    in_maps = _shard_inputs(x, weights)
    res = run_sharded(in_maps)
    full = np.concatenate(
        [res.results[c]["out"] for c in range(N_CORES)], axis=1
    ).astype(np.float32)
    full += bias[None, :]
    return full.reshape(B, IN_SIZE, OUT_SIZE)
